# revision 8
# baseline (speedup 1.0000x reference)
"""Trainium2 Bass kernel for the FFT-block (attention + conv FFN) problem.

Sharding: data-parallel over batch. B=16 items across 8 cores -> 2 items/core.
Each core runs the full block for its items; no collectives.

Per item:
  - attention via scores^T = K Q^T (softmax sums land on the partition axis and
    are folded into the ctx matmul through a ones-column appended to V); the
    per-head 1/Z normalization is broadcast across partitions with a K=1 PE
    matmul.  Attention matmuls run in fp32r (tf32-like, fp32 accumulate);
    softmax weights and V are bf16.
  - conv1 is 9 shifted bf16 matmuls over transposed activations hT [D, S_pad];
    conv2 runs 12 of its 16 cd-chunks as fp8e4m3 DoubleRow matmuls (2 chunks
    contracted per instruction, 2x PE throughput) and 4 chunks in bf16.  c1 is
    quantized to fp8 with a -0.5 shift (ReLU zeros land on an exact value);
    the shift is compensated host-side in the conv2 bias.  All conv2 products
    are scaled x128 (fp8 W2 range), folded out in the LN2 ReLU scale.
  - emission order software-pipelines item1's attention into item0's conv
    stream so the PE never drains.
"""
import sys, types
import numpy as np

B, S, D = 16, 1024, 512
H, DK = 8, 64
CD, KS = 2048, 9
EPS = 1e-5
NCORES = 8
NIT = B // NCORES
NDC = D // 128             # 4 d-chunks
NSC = S // 128             # 8 s-chunks
NCOL = S // 512            # 2 s-cols
NCD = CD // 128            # 16 cd-chunks
NF8P = 8                   # fp8 conv2 chunk-pairs (all 16 chunks)
NBF = NCD - 2 * NF8P       # bf16 conv2 chunks (12..15)
S2 = 128.0                 # fp8 W2 scale (all conv2 products carry x128)


def _install_ntff_hook():
    try:
        from antenv.axon_hooks import get_axon_ntff_profile_hook  # noqa
        return
    except ImportError:
        pass
    try:
        from trn_agent_boot.trn_boot import _ntff_profile_via_ctypes
        mod = types.ModuleType('antenv.axon_hooks')
        hook = _ntff_profile_via_ctypes('/opt/axon/libaxon_pjrt.so')
        mod.get_axon_ntff_profile_hook = lambda: hook
        sys.modules['antenv.axon_hooks'] = mod
    except Exception:
        pass


_BUILT = None


def _build():
    global _BUILT
    if _BUILT is not None:
        return _BUILT
    _install_ntff_hook()
    import concourse.bacc as bacc
    import concourse.mybir as mybir
    from concourse import tile
    from concourse.masks import make_identity
    from contextlib import ExitStack

    F32 = mybir.dt.float32
    F32R = mybir.dt.float32r
    BF16 = mybir.dt.bfloat16
    FP8 = mybir.dt.float8e4
    DR = mybir.MatmulPerfMode.DoubleRow
    AF = mybir.ActivationFunctionType
    ALU = mybir.AluOpType
    AX = mybir.AxisListType

    nc = bacc.Bacc("TRN2", target_bir_lowering=False, debug=False,
                   num_devices=NCORES)

    # ---- DRAM I/O (per core) ----
    d_xT = nc.dram_tensor("xT", [NIT, NDC, 128, S], BF16, kind="ExternalInput")
    d_xp = nc.dram_tensor("xp", [NIT, NSC, 128, D], F32, kind="ExternalInput")
    d_wqk = nc.dram_tensor("wqk", [2, 4, 128, 512], BF16, kind="ExternalInput")
    d_bqk = nc.dram_tensor("bqk", [128, 8], F32, kind="ExternalInput")
    d_wv = nc.dram_tensor("wv", [NDC, 128, 520], BF16, kind="ExternalInput")
    d_bvrow = nc.dram_tensor("bvrow", [128, 520], F32, kind="ExternalInput")
    d_wo = nc.dram_tensor("wo", [4, 128, 512], F32, kind="ExternalInput")
    d_w1 = nc.dram_tensor("w1", [NCD, NDC, 128, KS * 128], BF16,
                          kind="ExternalInput")
    d_w2f8 = nc.dram_tensor("w2f8", [NF8P, 128, 2, KS * 512], FP8,
                            kind="ExternalInput")
    d_w2bf = None
    if NBF > 0:
        d_w2bf = nc.dram_tensor("w2bf", [NBF, 128, KS * 512], BF16,
                                kind="ExternalInput")
    d_bc1s = nc.dram_tensor("bc1s", [128, NCD], F32, kind="ExternalInput")
    d_gb = nc.dram_tensor("gb", [5, 128, 512], F32, kind="ExternalInput")
    d_cones = nc.dram_tensor("cones", [128, 128], F32, kind="ExternalInput")
    d_czero = nc.dram_tensor("czero", [128, 8], BF16, kind="ExternalInput")
    d_cmh = nc.dram_tensor("cmh", [128, 2, 8], FP8, kind="ExternalInput")
    d_y = nc.dram_tensor("y", [NIT, NSC, 128, D], F32, kind="ExternalOutput")

    G1, B1, G2, B2, BC2 = range(5)

    with tile.TileContext(nc) as tc:
        est = ExitStack()
        with est:
            cp = est.enter_context(tc.tile_pool(name="const", bufs=1))
            pl = est.enter_context(tc.tile_pool(name="work", bufs=1))
            ps = est.enter_context(tc.tile_pool(name="psum", bufs=1, space="PSUM"))
            dp = est.enter_context(tc.tile_pool(name="dramp", bufs=1, space="DRAM"))

            h_dram = [[dp.tile([128, D], F32, tag=f"hd{it}_{sc}",
                               name=f"hd{it}_{sc}")
                       for sc in range(NSC)] for it in range(NIT)]

            # ---- constants ----
            t_bqk = cp.tile([128, 8], F32, tag="bqk")
            nc.sync.dma_start(t_bqk[:], d_bqk[:])
            t_gb = []
            for i in range(5):
                t = cp.tile([128, 512], F32, tag=f"gb{i}", name=f"gb{i}")
                t_gb.append(t)
            t_bc1s = cp.tile([128, NCD], F32, tag="bc1s")
            t_ident = cp.tile([128, 128], F32, tag="ident")
            make_identity(nc, t_ident[:])
            t_cones = cp.tile([128, 128], F32R, tag="cones")
            nc.sync.dma_start(t_cones[:], d_cones[:].bitcast(F32R))
            t_czero = cp.tile([128, 8], BF16, tag="czero")
            t_eps = cp.tile([128, 1], F32, tag="eps")
            nc.vector.memset(t_eps[:], EPS)
            t_wv = []
            for dc in range(NDC):
                t = cp.tile([128, 520], BF16, tag=f"wv{dc}", name=f"wv{dc}")
                nc.sync.dma_start(t[:], d_wv[dc])
                t_wv.append(t)
            t_bvrow = cp.tile([128, 520], F32R, tag="bvrow")
            nc.sync.dma_start(t_bvrow[:], d_bvrow[:].bitcast(F32R))
            t_wo = []
            for c in range(4):
                t = cp.tile([128, 512], F32R, tag=f"wo{c}", name=f"wo{c}")
                t_wo.append(t)

            def emit_late_consts():
                for i in range(5):
                    nc.sync.dma_start(t_gb[i][:], d_gb[i])
                nc.sync.dma_start(t_bc1s[:], d_bc1s[:])
                nc.sync.dma_start(t_czero[:], d_czero[:])
                for c in range(4):
                    nc.sync.dma_start(t_wo[c][:], d_wo[c].bitcast(F32R))

            # persistent hT tiles (bf16, padded s)
            hT = [[pl.tile([128, S + 8], BF16, tag=f"ht{it}_{dc}",
                           name=f"ht{it}_{dc}")
                   for dc in range(NDC)] for it in range(NIT)]

            state = [dict() for _ in range(NIT)]

            # ================= emit helpers =================
            def emit_x(it):
                st = state[it]
                xt = []
                for dc in range(NDC):
                    t = pl.tile([128, S], BF16, tag=f"xt{dc}", name=f"xt{dc}")
                    nc.sync.dma_start(t[:], d_xT[it, dc])
                    xt.append(t)
                st["xt"] = xt
                st["qkt"] = {}

            def emit_v(it, lo=0, hi=NSC):
                """V projection for one item (dense PE block)."""
                st = state[it]
                xt = st["xt"]
                vst = st.setdefault("vst", [None] * NSC)
                for tc_i in range(lo, hi):
                    vt = pl.tile([128, 520], BF16, tag=f"vst{tc_i}",
                                 bufs=2, name=f"vst{tc_i}")
                    for half in range(2):
                        colo = half * 260
                        pv = ps.tile([128, 260], F32, tag="pp", bufs=2)
                        for dc in range(NDC):
                            nc.tensor.matmul(
                                pv[:], xt[dc][:, tc_i * 128:(tc_i + 1) * 128],
                                t_wv[dc][:, colo:colo + 260],
                                start=(dc == 0), stop=False)
                        nc.tensor.matmul(
                            pv[:], t_cones[0:1, 0:128],
                            t_bvrow[0:1, colo:colo + 260],
                            start=False, stop=True)
                        nc.vector.tensor_copy(vt[:, colo:colo + 260], pv[:])
                    vst[tc_i] = vt

            def emit_qk(it, pair):
                st = state[it]
                xt = st["xt"]
                for proj in range(2):
                    wt = pl.tile([128, 512], BF16, tag=f"wqk{proj}",
                                 bufs=2, name="wt")
                    nc.sync.dma_start(wt[:], d_wqk[proj, pair])
                    qt = pl.tile([128, S], BF16, tag=f"qk{proj}{pair}",
                                 name="qt")
                    for scol in range(NCOL):
                        pq = ps.tile([128, 512], F32, tag="pp", bufs=2)
                        for dc in range(NDC):
                            nc.tensor.matmul(
                                pq[:], wt[:, dc * 128:(dc + 1) * 128],
                                xt[dc][:, scol * 512:(scol + 1) * 512],
                                start=(dc == 0), stop=(dc == NDC - 1))
                        nc.vector.tensor_scalar_add(
                            qt[:, scol * 512:(scol + 1) * 512], pq[:],
                            t_bqk[:, proj * 4 + pair:proj * 4 + pair + 1])
                    st["qkt"][(proj, pair)] = qt

            def emit_heads_pair(it, pair):
                st = state[it]
                if pair == 0:
                    st["ctxT"] = [pl.tile([128, S], F32R, tag=f"ct{c}",
                                          name=f"ct{c}") for c in range(4)]
                qT = st["qkt"][(0, pair)]
                kT = st["qkt"][(1, pair)]
                vst = st["vst"]
                ctxT = st["ctxT"]
                for sub in range(2):
                    h = 2 * pair + sub
                    hr = slice(sub * 64, sub * 64 + 64)
                    for scol in range(NCOL):
                        so = scol * 512
                        pex = []
                        for ti in range(NSC):
                            pp = ps.tile([128, 512], F32, tag="pp", bufs=2)
                            nc.tensor.matmul(
                                pp[:], kT[hr, ti * 128:(ti + 1) * 128],
                                qT[hr, so:so + 512], start=True, stop=True)
                            pe = pl.tile([128, 512], BF16, tag=f"pex{ti}",
                                         bufs=1, name="pe")
                            nc.scalar.activation(pe[:], pp[:], AF.Exp,
                                                 scale=0.125)
                            pex.append(pe)
                        pc = ps.tile([65, 512], F32, tag="pc", bufs=2)
                        for ti in range(NSC):
                            nc.tensor.matmul(
                                pc[:], vst[ti][:, h * 65:h * 65 + 65],
                                pex[ti][:], start=(ti == 0),
                                stop=(ti == NSC - 1))
                        zr = pl.tile([64, 512], F32R, tag="bcs", bufs=2,
                                     name="zr")
                        nc.vector.tensor_copy(zr[0:1, :], pc[64:65, :])
                        pb = ps.tile([64, 512], F32, tag="pp", bufs=2)
                        nc.tensor.matmul(pb[:], t_cones[0:1, 0:64], zr[0:1, :],
                                         start=True, stop=True)
                        bcs = pl.tile([64, 512], F32, tag="bcs", bufs=2,
                                      name="bcs")
                        nc.vector.reciprocal_approx_fast(out=bcs[:], in_=pb[:])
                        nc.vector.tensor_tensor(
                            ctxT[pair][hr, so:so + 512], pc[0:64, :],
                            bcs[:], ALU.mult)


            def emit_tail(it):
                """Wo + residual + LN1 + transpose into hT (+ h spill)."""
                st = state[it]
                ctxT = st["ctxT"]
                st_sum = pl.tile([128, NSC], F32, tag="st_sum", bufs=2)
                st_sq = pl.tile([128, NSC], F32, tag="st_sq", bufs=2)
                rr = []
                for sc in range(NSC):
                    xpt = pl.tile([128, 512], F32, tag="xpt", bufs=2)
                    nc.sync.dma_start(xpt[:], d_xp[it, sc])
                    pw = ps.tile([128, 512], F32, tag="pc", bufs=2)
                    for c in range(4):
                        nc.tensor.matmul(
                            pw[:], ctxT[c][:, sc * 128:(sc + 1) * 128],
                            t_wo[c][:], start=(c == 0), stop=(c == 3))
                    r = pl.tile([128, 512], F32, tag=f"res{sc}", name="r")
                    nc.vector.tensor_tensor(r[:], pw[:], xpt[:], ALU.add)
                    nc.vector.reduce_sum(st_sum[:, sc:sc + 1], r[:], axis=AX.X)
                    sq = pl.tile([128, 512], BF16, tag="sqs", bufs=2, name="sq")
                    nc.scalar.activation(sq[:], r[:], AF.Square,
                                         accum_out=st_sq[:, sc:sc + 1])
                    rr.append(r)
                mean8 = pl.tile([128, NSC], F32, tag="mean8", bufs=2)
                inv8 = pl.tile([128, NSC], F32, tag="inv8", bufs=2)
                msq = pl.tile([128, NSC], F32, tag="msq", bufs=2)
                nc.vector.tensor_scalar_mul(mean8[:], st_sum[:], 1.0 / D)
                nc.vector.tensor_scalar_mul(inv8[:], st_sq[:], 1.0 / D)
                nc.vector.tensor_tensor(msq[:], mean8[:], mean8[:], ALU.mult)
                nc.vector.tensor_tensor(inv8[:], inv8[:], msq[:], ALU.subtract)
                nc.scalar.activation(inv8[:], inv8[:], AF.Sqrt, bias=t_eps[:])
                nc.vector.reciprocal(inv8[:], inv8[:])
                for sc in range(NSC):
                    ht_ = pl.tile([128, 512], F32, tag="hst", bufs=2, name="h_")
                    nc.vector.tensor_scalar(
                        ht_[:], rr[sc][:], mean8[:, sc:sc + 1],
                        inv8[:, sc:sc + 1], ALU.subtract, ALU.mult)
                    nc.vector.tensor_tensor(ht_[:], ht_[:], t_gb[G1][:], ALU.mult)
                    nc.vector.tensor_tensor(ht_[:], ht_[:], t_gb[B1][:], ALU.add)
                    nc.sync.dma_start(h_dram[it][sc][:], ht_[:])
                    for dc in range(NDC):
                        pt = ps.tile([128, 128], F32, tag="pp", bufs=2)
                        nc.tensor.transpose(pt[:], ht_[:, dc * 128:(dc + 1) * 128],
                                            t_ident[:])
                        nc.scalar.copy(
                            hT[it][dc][:, 4 + sc * 128: 4 + (sc + 1) * 128],
                            pt[:])
                for dc in range(NDC):
                    nc.sync.dma_start(hT[it][dc][:, 0:4], d_czero[:, 0:4])
                    nc.sync.dma_start(hT[it][dc][:, S + 4:S + 8],
                                      d_czero[:, 4:8])

            o2 = [[None] * NSC for _ in range(NIT)]

            def emit_conv1_psum(it, cdc, w1t, scol):
                """36 bf16 matmuls of conv1 for (chunk, scol) into a psum."""
                pc1 = ps.tile([128, 512], F32, tag="c1p", bufs=2)
                idx = 0
                for k in range(KS):
                    for dc in range(NDC):
                        nc.tensor.matmul(
                            pc1[:], w1t[dc][:, k * 128:(k + 1) * 128],
                            hT[it][dc][:, scol * 512 + k:
                                       scol * 512 + k + 512],
                            start=(idx == 0), stop=(idx == 35))
                        idx += 1
                return pc1

            def load_w1(cdc):
                w1t = []
                for dc in range(NDC):
                    t = pl.tile([128, KS * 128], BF16, tag=f"w1t{dc}", bufs=2,
                                name="w1t")
                    nc.sync.dma_start(t[:], d_w1[cdc, dc])
                    w1t.append(t)
                return w1t

            def o2_acc(it, sc, pc2, first):
                if first:
                    t = pl.tile([128, 512], F32, tag=f"o2_{sc}",
                                name=f"o2_{sc}")
                    o2[it][sc] = t
                    nc.vector.tensor_copy(t[:], pc2[:])
                else:
                    nc.vector.tensor_tensor(o2[it][sc][:], pc2[:],
                                            o2[it][sc][:], ALU.add)

            ln2st = {}

            def emit_ln2_start(it):
                st_sum = pl.tile([128, NSC], F32, tag="st_sum", bufs=2)
                st_sq = pl.tile([128, NSC], F32, tag="st_sq", bufs=2)
                ln2st[it] = (st_sum, st_sq, [])

            def emit_ln2_pre(it, sc):
                st_sum, st_sq, rr = ln2st[it]
                t1 = pl.tile([128, 512], F32, tag="hst", bufs=2)
                nc.vector.tensor_tensor(t1[:], o2[it][sc][:], t_gb[BC2][:],
                                        ALU.add)
                nc.scalar.activation(t1[:], t1[:], AF.Relu, scale=1.0 / S2)
                hrl = pl.tile([128, 512], F32, tag="xpt", bufs=2)
                nc.sync.dma_start(hrl[:], h_dram[it][sc][:])
                r = pl.tile([128, 512], F32, tag=f"res{sc}", name="r2")
                nc.vector.tensor_tensor(r[:], t1[:], hrl[:], ALU.add)
                nc.vector.reduce_sum(st_sum[:, sc:sc + 1], r[:], axis=AX.X)
                sq = pl.tile([128, 512], BF16, tag="sqs", bufs=2, name="sq2")
                nc.scalar.activation(sq[:], r[:], AF.Square,
                                     accum_out=st_sq[:, sc:sc + 1])
                rr.append(r)

            def emit_ln2_post(it):
                st_sum, st_sq, rr = ln2st[it]
                mean8 = pl.tile([128, NSC], F32, tag="mean8", bufs=2)
                inv8 = pl.tile([128, NSC], F32, tag="inv8", bufs=2)
                msq = pl.tile([128, NSC], F32, tag="msq", bufs=2)
                nc.vector.tensor_scalar_mul(mean8[:], st_sum[:], 1.0 / D)
                nc.vector.tensor_scalar_mul(inv8[:], st_sq[:], 1.0 / D)
                nc.vector.tensor_tensor(msq[:], mean8[:], mean8[:], ALU.mult)
                nc.vector.tensor_tensor(inv8[:], inv8[:], msq[:], ALU.subtract)
                nc.scalar.activation(inv8[:], inv8[:], AF.Sqrt, bias=t_eps[:])
                nc.vector.reciprocal(inv8[:], inv8[:])
                for sc in range(NSC):
                    yt = pl.tile([128, 512], F32, tag="hst", bufs=2)
                    nc.vector.tensor_scalar(
                        yt[:], rr[sc][:], mean8[:, sc:sc + 1],
                        inv8[:, sc:sc + 1], ALU.subtract, ALU.mult)
                    nc.vector.tensor_tensor(yt[:], yt[:], t_gb[G2][:], ALU.mult)
                    nc.vector.tensor_tensor(yt[:], yt[:], t_gb[B2][:], ALU.add)
                    nc.sync.dma_start(d_y[it, sc], yt[:])

            def emit_conv_pair(it, p):
                """fp8 conv2 pair unit: chunks (2p, 2p+1)."""
                w2t = pl.tile([128, 2, KS * 512], FP8, tag="w2t", bufs=2,
                              name="w2t")
                nc.sync.dma_start(w2t[:], d_w2f8[p])
                c1d = pl.tile([128, 2, S + 16], FP8, tag="c1d", bufs=2,
                              name="c1d")
                nc.sync.dma_start(c1d[:, :, 0:4], d_cmh[:, :, 0:4])
                nc.sync.dma_start(c1d[:, :, S + 4:S + 8], d_cmh[:, :, 4:8])
                for slot in range(2):
                    cdc = 2 * p + slot
                    w1t = load_w1(cdc)
                    for scol in range(NCOL):
                        pc1 = emit_conv1_psum(it, cdc, w1t, scol)
                        tmp = pl.tile([128, 512], BF16, tag="c1tmp", bufs=2,
                                      name="c1tmp")
                        nc.scalar.activation(
                            tmp[:], pc1[:], AF.Relu,
                            bias=t_bc1s[:, cdc:cdc + 1])
                        nc.vector.tensor_scalar_add(
                            c1d[:, slot, 4 + scol * 512: 4 + (scol + 1) * 512],
                            tmp[:], -0.5)
                last = (p == NF8P - 1)
                if last:
                    emit_ln2_start(it)
                for sc in range(NSC):
                    pc2 = ps.tile([128, 512], F32, tag="c2p", bufs=2)
                    for k in range(KS):
                        nc.tensor.matmul(
                            pc2[:], c1d[:, :, sc * 128 + k: sc * 128 + k + 128],
                            w2t[:, :, k * 512:(k + 1) * 512],
                            start=(k == 0), stop=(k == KS - 1), perf_mode=DR)
                    o2_acc(it, sc, pc2, p == 0)
                    if last:
                        emit_ln2_pre(it, sc)

            def emit_conv_bf(it, j):
                """bf16 conv2 single-chunk unit: chunk 12+j."""
                cdc = 2 * NF8P + j
                w2t = pl.tile([128, KS * 512], BF16, tag="w2t", bufs=2,
                              name="w2tb")
                nc.sync.dma_start(w2t[:], d_w2bf[j])
                w1t = load_w1(cdc)
                c1t = pl.tile([128, S + 8], BF16, tag="c1d", bufs=2, name="c1t")
                nc.sync.dma_start(c1t[:, 0:4], d_czero[:, 0:4])
                nc.sync.dma_start(c1t[:, S + 4:S + 8], d_czero[:, 4:8])
                for scol in range(NCOL):
                    pc1 = emit_conv1_psum(it, cdc, w1t, scol)
                    nc.scalar.activation(
                        c1t[:, 4 + scol * 512: 4 + (scol + 1) * 512],
                        pc1[:], AF.Relu, bias=t_bc1s[:, cdc:cdc + 1])
                for sc in range(NSC):
                    pc2 = ps.tile([128, 512], F32, tag="c2p", bufs=2)
                    for k in range(KS):
                        nc.tensor.matmul(
                            pc2[:], c1t[:, sc * 128 + k: sc * 128 + k + 128],
                            w2t[:, k * 512:(k + 1) * 512],
                            start=(k == 0), stop=(k == KS - 1))
                    o2_acc(it, sc, pc2, False)

            def emit_conv_unit(it, u):
                if u < NF8P:
                    emit_conv_pair(it, u)
                else:
                    emit_conv_bf(it, u - NF8P)

            # ================= emission order =================
            NU = NF8P + NBF
            emit_x(0)
            emit_v(0)
            for pair in range(4):
                emit_qk(0, pair)
            emit_x(1)
            emit_late_consts()
            for pair in range(4):
                emit_heads_pair(0, pair)
                emit_qk(1, pair)
                emit_v(1, 2 * pair, 2 * pair + 2)
            emit_tail(0)
            for u in range(NU):
                emit_conv_unit(0, u)
                if u < 4:
                    emit_heads_pair(1, u)
                elif u == 5:
                    emit_tail(1)
            emit_ln2_post(0)
            for u in range(NU):
                emit_conv_unit(1, u)
            emit_ln2_post(1)

    nc.compile()
    _BUILT = nc
    return nc


def _prep_host(inputs):
    import ml_dtypes
    bf16 = ml_dtypes.bfloat16
    fp8 = ml_dtypes.float8_e4m3
    x = np.asarray(inputs["x"], np.float32)
    Wq = np.asarray(inputs["Wq"], np.float32)
    bq = np.asarray(inputs["bq"], np.float32)
    Wk = np.asarray(inputs["Wk"], np.float32)
    bk = np.asarray(inputs["bk"], np.float32)
    Wv = np.asarray(inputs["Wv"], np.float32)
    bv = np.asarray(inputs["bv"], np.float32)
    Wo = np.asarray(inputs["Wo"], np.float32)
    bo = np.asarray(inputs["bo"], np.float32)
    g1 = np.asarray(inputs["g1"], np.float32)
    b1 = np.asarray(inputs["b1"], np.float32)
    g2 = np.asarray(inputs["g2"], np.float32)
    b2 = np.asarray(inputs["b2"], np.float32)
    Wc1 = np.asarray(inputs["Wc1"], np.float32)
    bc1 = np.asarray(inputs["bc1"], np.float32)
    Wc2 = np.asarray(inputs["Wc2"], np.float32)
    bc2 = np.asarray(inputs["bc2"], np.float32)

    xT = np.ascontiguousarray(x.transpose(0, 2, 1).reshape(B, NDC, 128, S)).astype(bf16)
    xp = np.ascontiguousarray((x + bo[None, None, :]).reshape(B, NSC, 128, D))

    wqk = np.zeros((2, 4, 128, 512), np.float32)  # cast to bf16 below
    for proj, W in ((0, Wq), (1, Wk)):
        for pair in range(4):
            blk = np.concatenate([W[2 * pair], W[2 * pair + 1]], axis=1)
            wqk[proj, pair] = blk.reshape(NDC, 128, 128).transpose(1, 0, 2) \
                                 .reshape(128, 512)
    bqk = np.zeros((128, 8), np.float32)
    for proj, b in ((0, bq), (1, bk)):
        for pair in range(4):
            bqk[:, proj * 4 + pair] = np.concatenate(
                [b[2 * pair], b[2 * pair + 1]])

    wv = np.zeros((NDC, 128, 520), np.float32)
    bvrow = np.zeros((128, 520), np.float32)
    for h in range(H):
        wv[:, :, h * 65:h * 65 + 64] = Wv[h].reshape(NDC, 128, 64)
        bvrow[0, h * 65:h * 65 + 64] = bv[h]
        bvrow[0, h * 65 + 64] = 1.0

    wo = np.ascontiguousarray(Wo.reshape(4, 128, 512))

    w1 = np.ascontiguousarray(
        Wc1.reshape(NCD, 128, NDC, 128, KS).transpose(0, 2, 3, 4, 1)
           .reshape(NCD, NDC, 128, KS * 128)).astype(bf16)
    # conv2 weights: fp8 pairs (chunks 0..11, x128) + bf16 (12..15, x128)
    w2full = Wc2.reshape(D, NCD, 128, KS).transpose(1, 2, 3, 0) \
                .reshape(NCD, 128, KS * 512) * S2
    w2f8 = np.ascontiguousarray(
        w2full[:2 * NF8P].reshape(NF8P, 2, 128, KS * 512)
        .transpose(0, 2, 1, 3)).astype(fp8)
    bc1s = np.ascontiguousarray(bc1.reshape(NCD, 128).T)

    # conv2 bias, prescaled by S2, + compensation for the -0.5 c1 shift on
    # the fp8 channels (0.5 * sum of their W2 taps)
    comp = 0.5 * Wc2[:, :2 * NF8P * 128, :].sum(axis=(1, 2))
    bc2s = S2 * (bc2 + comp)

    gb = np.stack([np.tile(v[None, :], (128, 1))
                   for v in (g1, b1, g2, b2, bc2s)]).astype(np.float32)
    cones = np.ones((128, 128), np.float32)
    czero = np.zeros((128, 8), bf16)
    cmh = np.full((128, 2, 8), -0.5, fp8)

    wqk = wqk.astype(bf16)
    wv = wv.astype(bf16)
    shared = dict(wqk=wqk, bqk=bqk, wv=wv, bvrow=bvrow, wo=wo,
                  w1=w1, w2f8=w2f8, bc1s=bc1s, gb=gb,
                  cones=cones, czero=czero, cmh=cmh)
    if NBF > 0:
        shared["w2bf"] = np.ascontiguousarray(w2full[2 * NF8P:]).astype(bf16)
    in_maps = []
    for c in range(NCORES):
        m = dict(shared)
        m["xT"] = np.ascontiguousarray(xT[c * NIT:(c + 1) * NIT])
        m["xp"] = np.ascontiguousarray(xp[c * NIT:(c + 1) * NIT])
        in_maps.append(m)
    return in_maps


def run(inputs, trace=False, **trace_kwargs):
    nc = _build()
    from concourse.bass_utils import run_bass_kernel_spmd
    in_maps = _prep_host(inputs)
    res = run_bass_kernel_spmd(nc, in_maps, core_ids=list(range(NCORES)),
                               trace=trace, **trace_kwargs)
    y = np.concatenate([res.results[c]["y"].reshape(NIT, S, D)
                        for c in range(NCORES)], axis=0)
    return y, res


def kernel(**inputs):
    y, _ = run(inputs, trace=False)
    return y


# revision 9
# speedup vs baseline: 1.0169x; 1.0169x over previous
"""Trainium2 Bass kernel for the FFT-block (attention + conv FFN) problem.

Sharding: data-parallel over batch. B=16 items across 8 cores -> 2 items/core.
Each core runs the full block for its items; no collectives.

Per item:
  - attention via scores^T = K Q^T (softmax sums land on the partition axis and
    are folded into the ctx matmul through a ones-column appended to V); the
    per-head 1/Z normalization is broadcast across partitions with a K=1 PE
    matmul.  Attention matmuls run in fp32r (tf32-like, fp32 accumulate);
    softmax weights and V are bf16.
  - conv1 is 9 shifted bf16 matmuls over transposed activations hT [D, S_pad];
    conv2 runs 12 of its 16 cd-chunks as fp8e4m3 DoubleRow matmuls (2 chunks
    contracted per instruction, 2x PE throughput) and 4 chunks in bf16.  c1 is
    quantized to fp8 with a -0.5 shift (ReLU zeros land on an exact value);
    the shift is compensated host-side in the conv2 bias.  All conv2 products
    are scaled x128 (fp8 W2 range), folded out in the LN2 ReLU scale.
  - emission order software-pipelines item1's attention into item0's conv
    stream so the PE never drains.
"""
import sys, types
import numpy as np

B, S, D = 16, 1024, 512
H, DK = 8, 64
CD, KS = 2048, 9
EPS = 1e-5
NCORES = 8
NIT = B // NCORES
NDC = D // 128             # 4 d-chunks
NSC = S // 128             # 8 s-chunks
NCOL = S // 512            # 2 s-cols
NCD = CD // 128            # 16 cd-chunks
NF8P = 8                   # fp8 conv2 chunk-pairs (all 16 chunks)
NBF = NCD - 2 * NF8P       # bf16 conv2 chunks (12..15)
S2 = 128.0                 # fp8 W2 scale (all conv2 products carry x128)


def _install_ntff_hook():
    try:
        from antenv.axon_hooks import get_axon_ntff_profile_hook  # noqa
        return
    except ImportError:
        pass
    try:
        from trn_agent_boot.trn_boot import _ntff_profile_via_ctypes
        mod = types.ModuleType('antenv.axon_hooks')
        hook = _ntff_profile_via_ctypes('/opt/axon/libaxon_pjrt.so')
        mod.get_axon_ntff_profile_hook = lambda: hook
        sys.modules['antenv.axon_hooks'] = mod
    except Exception:
        pass


_BUILT = None


def _build():
    global _BUILT
    if _BUILT is not None:
        return _BUILT
    _install_ntff_hook()
    import concourse.bacc as bacc
    import concourse.mybir as mybir
    from concourse import tile
    from concourse.masks import make_identity
    from contextlib import ExitStack

    F32 = mybir.dt.float32
    F32R = mybir.dt.float32r
    BF16 = mybir.dt.bfloat16
    FP8 = mybir.dt.float8e4
    DR = mybir.MatmulPerfMode.DoubleRow
    AF = mybir.ActivationFunctionType
    ALU = mybir.AluOpType
    AX = mybir.AxisListType

    nc = bacc.Bacc("TRN2", target_bir_lowering=False, debug=False,
                   num_devices=NCORES)

    # ---- DRAM I/O (per core) ----
    d_xT = nc.dram_tensor("xT", [NIT, NDC, 128, S], BF16, kind="ExternalInput")
    d_xp = nc.dram_tensor("xp", [NIT, NSC, 128, D], F32, kind="ExternalInput")
    d_wqk = nc.dram_tensor("wqk", [2, 4, 128, 512], BF16, kind="ExternalInput")
    d_bqk = nc.dram_tensor("bqk", [128, 8], F32, kind="ExternalInput")
    d_wv = nc.dram_tensor("wv", [NDC, 128, 520], BF16, kind="ExternalInput")
    d_bvrow = nc.dram_tensor("bvrow", [128, 520], F32, kind="ExternalInput")
    d_wo = nc.dram_tensor("wo", [4, 128, 512], F32, kind="ExternalInput")
    d_w1 = nc.dram_tensor("w1", [NCD, NDC, 128, KS * 128], BF16,
                          kind="ExternalInput")
    d_w2f8 = nc.dram_tensor("w2f8", [NF8P, 128, 2, KS * 512], FP8,
                            kind="ExternalInput")
    d_w2bf = None
    if NBF > 0:
        d_w2bf = nc.dram_tensor("w2bf", [NBF, 128, KS * 512], BF16,
                                kind="ExternalInput")
    d_bc1s = nc.dram_tensor("bc1s", [128, NCD], F32, kind="ExternalInput")
    d_gb = nc.dram_tensor("gb", [5, 128, 512], F32, kind="ExternalInput")
    d_cones = nc.dram_tensor("cones", [128, 128], F32, kind="ExternalInput")
    d_czero = nc.dram_tensor("czero", [128, 8], BF16, kind="ExternalInput")
    d_cmh = nc.dram_tensor("cmh", [128, 2, 8], FP8, kind="ExternalInput")
    d_y = nc.dram_tensor("y", [NIT, NSC, 128, D], F32, kind="ExternalOutput")

    G1, B1, G2, B2, BC2 = range(5)

    with tile.TileContext(nc) as tc:
        est = ExitStack()
        with est:
            cp = est.enter_context(tc.tile_pool(name="const", bufs=1))
            pl = est.enter_context(tc.tile_pool(name="work", bufs=1))
            ps = est.enter_context(tc.tile_pool(name="psum", bufs=1, space="PSUM"))
            dp = est.enter_context(tc.tile_pool(name="dramp", bufs=1, space="DRAM"))

            h_dram = [[dp.tile([128, D], F32, tag=f"hd{it}_{sc}",
                               name=f"hd{it}_{sc}")
                       for sc in range(NSC)] for it in range(NIT)]

            # ---- constants ----
            t_bqk = cp.tile([128, 8], F32, tag="bqk")
            nc.sync.dma_start(t_bqk[:], d_bqk[:])
            t_gb = []
            for i in range(5):
                t = cp.tile([128, 512], F32, tag=f"gb{i}", name=f"gb{i}")
                t_gb.append(t)
            t_bc1s = cp.tile([128, NCD], F32, tag="bc1s")
            t_ident = cp.tile([128, 128], F32, tag="ident")
            make_identity(nc, t_ident[:])
            t_cones = cp.tile([128, 128], F32R, tag="cones")
            nc.sync.dma_start(t_cones[:], d_cones[:].bitcast(F32R))
            t_czero = cp.tile([128, 8], BF16, tag="czero")
            t_eps = cp.tile([128, 1], F32, tag="eps")
            nc.vector.memset(t_eps[:], EPS)
            t_wv = []
            for dc in range(NDC):
                t = cp.tile([128, 520], BF16, tag=f"wv{dc}", name=f"wv{dc}")
                nc.sync.dma_start(t[:], d_wv[dc])
                t_wv.append(t)
            t_bvrow = cp.tile([128, 520], F32R, tag="bvrow")
            nc.sync.dma_start(t_bvrow[:], d_bvrow[:].bitcast(F32R))
            t_wo = []
            for c in range(4):
                t = cp.tile([128, 512], F32R, tag=f"wo{c}", name=f"wo{c}")
                t_wo.append(t)

            def emit_late_consts():
                for i in range(5):
                    nc.sync.dma_start(t_gb[i][:], d_gb[i])
                nc.sync.dma_start(t_bc1s[:], d_bc1s[:])
                nc.sync.dma_start(t_czero[:], d_czero[:])
                for c in range(4):
                    nc.sync.dma_start(t_wo[c][:], d_wo[c].bitcast(F32R))

            # persistent hT tiles (bf16, padded s)
            hT = [[pl.tile([128, S + 8], BF16, tag=f"ht{it}_{dc}",
                           name=f"ht{it}_{dc}")
                   for dc in range(NDC)] for it in range(NIT)]

            state = [dict() for _ in range(NIT)]

            # ================= emit helpers =================
            def emit_x(it):
                st = state[it]
                xt = []
                for dc in range(NDC):
                    t = pl.tile([128, S], BF16, tag=f"xt{dc}", name=f"xt{dc}")
                    nc.sync.dma_start(t[:], d_xT[it, dc])
                    xt.append(t)
                st["xt"] = xt
                st["qkt"] = {}

            def emit_v(it, lo=0, hi=NSC):
                """V projection for one item (dense PE block)."""
                st = state[it]
                xt = st["xt"]
                vst = st.setdefault("vst", [None] * NSC)
                for tc_i in range(lo, hi):
                    vt = pl.tile([128, 520], BF16, tag=f"vst{tc_i}",
                                 bufs=2, name=f"vst{tc_i}")
                    for half in range(2):
                        colo = half * 260
                        pv = ps.tile([128, 260], F32, tag="pp", bufs=2)
                        for dc in range(NDC):
                            nc.tensor.matmul(
                                pv[:], xt[dc][:, tc_i * 128:(tc_i + 1) * 128],
                                t_wv[dc][:, colo:colo + 260],
                                start=(dc == 0), stop=False)
                        nc.tensor.matmul(
                            pv[:], t_cones[0:1, 0:128],
                            t_bvrow[0:1, colo:colo + 260],
                            start=False, stop=True)
                        nc.vector.tensor_copy(vt[:, colo:colo + 260], pv[:])
                    vst[tc_i] = vt

            def emit_qk(it, pair):
                st = state[it]
                xt = st["xt"]
                for proj in range(2):
                    wt = pl.tile([128, 512], BF16, tag=f"wqk{proj}",
                                 bufs=2, name="wt")
                    nc.sync.dma_start(wt[:], d_wqk[proj, pair])
                    qt = pl.tile([128, S], BF16, tag=f"qk{proj}{pair}",
                                 name="qt")
                    for scol in range(NCOL):
                        pq = ps.tile([128, 512], F32, tag="pp", bufs=2)
                        for dc in range(NDC):
                            nc.tensor.matmul(
                                pq[:], wt[:, dc * 128:(dc + 1) * 128],
                                xt[dc][:, scol * 512:(scol + 1) * 512],
                                start=(dc == 0), stop=(dc == NDC - 1))
                        nc.vector.tensor_scalar_add(
                            qt[:, scol * 512:(scol + 1) * 512], pq[:],
                            t_bqk[:, proj * 4 + pair:proj * 4 + pair + 1])
                    st["qkt"][(proj, pair)] = qt

            def _finish_block(it, fb):
                pc, pair, hr, so = fb
                ctxT = state[it]["ctxT"]
                zr = pl.tile([64, 512], F32R, tag="bcs", bufs=2,
                             name="zr")
                nc.vector.tensor_copy(zr[0:1, :], pc[64:65, :])
                pb = ps.tile([64, 512], F32, tag="pp", bufs=2)
                nc.tensor.matmul(pb[:], t_cones[0:1, 0:64], zr[0:1, :],
                                 start=True, stop=True)
                bcs = pl.tile([64, 512], F32, tag="bcs", bufs=2,
                              name="bcs")
                nc.vector.reciprocal_approx_fast(out=bcs[:], in_=pb[:])
                nc.vector.tensor_tensor(
                    ctxT[pair][hr, so:so + 512], pc[0:64, :],
                    bcs[:], ALU.mult)

            def emit_heads_pair(it, pair):
                st = state[it]
                if pair == 0:
                    st["ctxT"] = [pl.tile([128, S], F32R, tag=f"ct{c}",
                                          name=f"ct{c}") for c in range(4)]
                    st["pend"] = None
                qT = st["qkt"][(0, pair)]
                kT = st["qkt"][(1, pair)]
                vst = st["vst"]
                for sub in range(2):
                    h = 2 * pair + sub
                    hr = slice(sub * 64, sub * 64 + 64)
                    for scol in range(NCOL):
                        so = scol * 512
                        pex = []
                        for ti in range(NSC):
                            pp = ps.tile([128, 512], F32, tag="pp", bufs=2)
                            nc.tensor.matmul(
                                pp[:], kT[hr, ti * 128:(ti + 1) * 128],
                                qT[hr, so:so + 512], start=True, stop=True)
                            pe = pl.tile([128, 512], BF16, tag=f"pex{ti}",
                                         bufs=1, name="pe")
                            nc.scalar.activation(pe[:], pp[:], AF.Exp,
                                                 scale=0.125)
                            pex.append(pe)
                        pc = ps.tile([65, 512], F32, tag="pc", bufs=2)
                        for ti in range(NSC):
                            nc.tensor.matmul(
                                pc[:], vst[ti][:, h * 65:h * 65 + 65],
                                pex[ti][:], start=(ti == 0),
                                stop=(ti == NSC - 1))
                        if st["pend"] is not None:
                            _finish_block(it, st["pend"])
                        st["pend"] = (pc, pair, hr, so)
                if pair == 3:
                    _finish_block(it, st["pend"])
                    st["pend"] = None


            def emit_tail(it):
                """Wo + residual + LN1 + transpose into hT (+ h spill)."""
                st = state[it]
                ctxT = st["ctxT"]
                st_sum = pl.tile([128, NSC], F32, tag="st_sum", bufs=2)
                st_sq = pl.tile([128, NSC], F32, tag="st_sq", bufs=2)
                rr = []
                for sc in range(NSC):
                    xpt = pl.tile([128, 512], F32, tag="xpt", bufs=2)
                    nc.sync.dma_start(xpt[:], d_xp[it, sc])
                    pw = ps.tile([128, 512], F32, tag="pc", bufs=2)
                    for c in range(4):
                        nc.tensor.matmul(
                            pw[:], ctxT[c][:, sc * 128:(sc + 1) * 128],
                            t_wo[c][:], start=(c == 0), stop=(c == 3))
                    r = pl.tile([128, 512], F32, tag=f"res{sc}", name="r")
                    nc.vector.tensor_tensor(r[:], pw[:], xpt[:], ALU.add)
                    nc.vector.reduce_sum(st_sum[:, sc:sc + 1], r[:], axis=AX.X)
                    sq = pl.tile([128, 512], BF16, tag="sqs", bufs=2, name="sq")
                    nc.scalar.activation(sq[:], r[:], AF.Square,
                                         accum_out=st_sq[:, sc:sc + 1])
                    rr.append(r)
                mean8 = pl.tile([128, NSC], F32, tag="mean8", bufs=2)
                inv8 = pl.tile([128, NSC], F32, tag="inv8", bufs=2)
                msq = pl.tile([128, NSC], F32, tag="msq", bufs=2)
                nc.vector.tensor_scalar_mul(mean8[:], st_sum[:], 1.0 / D)
                nc.vector.tensor_scalar_mul(inv8[:], st_sq[:], 1.0 / D)
                nc.vector.tensor_tensor(msq[:], mean8[:], mean8[:], ALU.mult)
                nc.vector.tensor_tensor(inv8[:], inv8[:], msq[:], ALU.subtract)
                nc.scalar.activation(inv8[:], inv8[:], AF.Sqrt, bias=t_eps[:])
                nc.vector.reciprocal(inv8[:], inv8[:])
                for sc in range(NSC):
                    ht_ = pl.tile([128, 512], F32, tag="hst", bufs=2, name="h_")
                    nc.vector.tensor_scalar(
                        ht_[:], rr[sc][:], mean8[:, sc:sc + 1],
                        inv8[:, sc:sc + 1], ALU.subtract, ALU.mult)
                    nc.vector.tensor_tensor(ht_[:], ht_[:], t_gb[G1][:], ALU.mult)
                    nc.vector.tensor_tensor(ht_[:], ht_[:], t_gb[B1][:], ALU.add)
                    nc.sync.dma_start(h_dram[it][sc][:], ht_[:])
                    for dc in range(NDC):
                        pt = ps.tile([128, 128], F32, tag="pp", bufs=2)
                        nc.tensor.transpose(pt[:], ht_[:, dc * 128:(dc + 1) * 128],
                                            t_ident[:])
                        nc.scalar.copy(
                            hT[it][dc][:, 4 + sc * 128: 4 + (sc + 1) * 128],
                            pt[:])
                for dc in range(NDC):
                    nc.sync.dma_start(hT[it][dc][:, 0:4], d_czero[:, 0:4])
                    nc.sync.dma_start(hT[it][dc][:, S + 4:S + 8],
                                      d_czero[:, 4:8])

            o2 = [[None] * NSC for _ in range(NIT)]

            def emit_conv1_psum(it, cdc, w1t, scol):
                """36 bf16 matmuls of conv1 for (chunk, scol) into a psum."""
                pc1 = ps.tile([128, 512], F32, tag="c1p", bufs=2)
                idx = 0
                for k in range(KS):
                    for dc in range(NDC):
                        nc.tensor.matmul(
                            pc1[:], w1t[dc][:, k * 128:(k + 1) * 128],
                            hT[it][dc][:, scol * 512 + k:
                                       scol * 512 + k + 512],
                            start=(idx == 0), stop=(idx == 35))
                        idx += 1
                return pc1

            def load_w1(cdc):
                w1t = []
                for dc in range(NDC):
                    t = pl.tile([128, KS * 128], BF16, tag=f"w1t{dc}", bufs=2,
                                name="w1t")
                    nc.sync.dma_start(t[:], d_w1[cdc, dc])
                    w1t.append(t)
                return w1t

            def o2_acc(it, sc, pc2, first):
                if first:
                    t = pl.tile([128, 512], F32, tag=f"o2_{sc}",
                                name=f"o2_{sc}")
                    o2[it][sc] = t
                    nc.vector.tensor_copy(t[:], pc2[:])
                else:
                    nc.vector.tensor_tensor(o2[it][sc][:], pc2[:],
                                            o2[it][sc][:], ALU.add)

            ln2st = {}

            def emit_ln2_start(it):
                st_sum = pl.tile([128, NSC], F32, tag="st_sum", bufs=2)
                st_sq = pl.tile([128, NSC], F32, tag="st_sq", bufs=2)
                ln2st[it] = (st_sum, st_sq, [])

            def emit_ln2_pre(it, sc):
                st_sum, st_sq, rr = ln2st[it]
                t1 = pl.tile([128, 512], F32, tag="hst", bufs=2)
                nc.vector.tensor_tensor(t1[:], o2[it][sc][:], t_gb[BC2][:],
                                        ALU.add)
                nc.scalar.activation(t1[:], t1[:], AF.Relu, scale=1.0 / S2)
                hrl = pl.tile([128, 512], F32, tag="xpt", bufs=2)
                nc.sync.dma_start(hrl[:], h_dram[it][sc][:])
                r = pl.tile([128, 512], F32, tag=f"res{sc}", name="r2")
                nc.vector.tensor_tensor(r[:], t1[:], hrl[:], ALU.add)
                nc.vector.reduce_sum(st_sum[:, sc:sc + 1], r[:], axis=AX.X)
                sq = pl.tile([128, 512], BF16, tag="sqs", bufs=2, name="sq2")
                nc.scalar.activation(sq[:], r[:], AF.Square,
                                     accum_out=st_sq[:, sc:sc + 1])
                rr.append(r)

            def emit_ln2_post(it):
                st_sum, st_sq, rr = ln2st[it]
                mean8 = pl.tile([128, NSC], F32, tag="mean8", bufs=2)
                inv8 = pl.tile([128, NSC], F32, tag="inv8", bufs=2)
                msq = pl.tile([128, NSC], F32, tag="msq", bufs=2)
                nc.vector.tensor_scalar_mul(mean8[:], st_sum[:], 1.0 / D)
                nc.vector.tensor_scalar_mul(inv8[:], st_sq[:], 1.0 / D)
                nc.vector.tensor_tensor(msq[:], mean8[:], mean8[:], ALU.mult)
                nc.vector.tensor_tensor(inv8[:], inv8[:], msq[:], ALU.subtract)
                nc.scalar.activation(inv8[:], inv8[:], AF.Sqrt, bias=t_eps[:])
                nc.vector.reciprocal(inv8[:], inv8[:])
                for sc in range(NSC):
                    yt = pl.tile([128, 512], F32, tag="hst", bufs=2)
                    nc.vector.tensor_scalar(
                        yt[:], rr[sc][:], mean8[:, sc:sc + 1],
                        inv8[:, sc:sc + 1], ALU.subtract, ALU.mult)
                    nc.vector.tensor_tensor(yt[:], yt[:], t_gb[G2][:], ALU.mult)
                    nc.vector.tensor_tensor(yt[:], yt[:], t_gb[B2][:], ALU.add)
                    nc.sync.dma_start(d_y[it, sc], yt[:])

            def emit_conv_pair(it, p):
                """fp8 conv2 pair unit: chunks (2p, 2p+1)."""
                w2t = pl.tile([128, 2, KS * 512], FP8, tag="w2t", bufs=2,
                              name="w2t")
                nc.sync.dma_start(w2t[:], d_w2f8[p])
                c1d = pl.tile([128, 2, S + 16], FP8, tag="c1d", bufs=2,
                              name="c1d")
                nc.sync.dma_start(c1d[:, :, 0:4], d_cmh[:, :, 0:4])
                nc.sync.dma_start(c1d[:, :, S + 4:S + 8], d_cmh[:, :, 4:8])
                for slot in range(2):
                    cdc = 2 * p + slot
                    w1t = load_w1(cdc)
                    for scol in range(NCOL):
                        pc1 = emit_conv1_psum(it, cdc, w1t, scol)
                        tmp = pl.tile([128, 512], BF16, tag="c1tmp", bufs=2,
                                      name="c1tmp")
                        nc.scalar.activation(
                            tmp[:], pc1[:], AF.Relu,
                            bias=t_bc1s[:, cdc:cdc + 1])
                        nc.vector.tensor_scalar_add(
                            c1d[:, slot, 4 + scol * 512: 4 + (scol + 1) * 512],
                            tmp[:], -0.5)
                last = (p == NF8P - 1)
                if last:
                    emit_ln2_start(it)
                for sc in range(NSC):
                    pc2 = ps.tile([128, 512], F32, tag="c2p", bufs=2)
                    for k in range(KS):
                        nc.tensor.matmul(
                            pc2[:], c1d[:, :, sc * 128 + k: sc * 128 + k + 128],
                            w2t[:, :, k * 512:(k + 1) * 512],
                            start=(k == 0), stop=(k == KS - 1), perf_mode=DR)
                    o2_acc(it, sc, pc2, p == 0)
                    if last:
                        emit_ln2_pre(it, sc)

            def emit_conv_bf(it, j):
                """bf16 conv2 single-chunk unit: chunk 12+j."""
                cdc = 2 * NF8P + j
                w2t = pl.tile([128, KS * 512], BF16, tag="w2t", bufs=2,
                              name="w2tb")
                nc.sync.dma_start(w2t[:], d_w2bf[j])
                w1t = load_w1(cdc)
                c1t = pl.tile([128, S + 8], BF16, tag="c1d", bufs=2, name="c1t")
                nc.sync.dma_start(c1t[:, 0:4], d_czero[:, 0:4])
                nc.sync.dma_start(c1t[:, S + 4:S + 8], d_czero[:, 4:8])
                for scol in range(NCOL):
                    pc1 = emit_conv1_psum(it, cdc, w1t, scol)
                    nc.scalar.activation(
                        c1t[:, 4 + scol * 512: 4 + (scol + 1) * 512],
                        pc1[:], AF.Relu, bias=t_bc1s[:, cdc:cdc + 1])
                for sc in range(NSC):
                    pc2 = ps.tile([128, 512], F32, tag="c2p", bufs=2)
                    for k in range(KS):
                        nc.tensor.matmul(
                            pc2[:], c1t[:, sc * 128 + k: sc * 128 + k + 128],
                            w2t[:, k * 512:(k + 1) * 512],
                            start=(k == 0), stop=(k == KS - 1))
                    o2_acc(it, sc, pc2, False)

            def emit_conv_unit(it, u):
                if u < NF8P:
                    emit_conv_pair(it, u)
                else:
                    emit_conv_bf(it, u - NF8P)

            # ================= emission order =================
            NU = NF8P + NBF
            emit_x(0)
            emit_v(0)
            for pair in range(4):
                emit_qk(0, pair)
            emit_x(1)
            emit_late_consts()
            for pair in range(4):
                emit_heads_pair(0, pair)
                emit_qk(1, pair)
                emit_v(1, 2 * pair, 2 * pair + 2)
            emit_tail(0)
            for u in range(NU):
                emit_conv_unit(0, u)
                if u < 4:
                    emit_heads_pair(1, u)
                elif u == 5:
                    emit_tail(1)
            emit_ln2_post(0)
            for u in range(NU):
                emit_conv_unit(1, u)
            emit_ln2_post(1)

    nc.compile()
    _BUILT = nc
    return nc


def _prep_host(inputs):
    import ml_dtypes
    bf16 = ml_dtypes.bfloat16
    fp8 = ml_dtypes.float8_e4m3
    x = np.asarray(inputs["x"], np.float32)
    Wq = np.asarray(inputs["Wq"], np.float32)
    bq = np.asarray(inputs["bq"], np.float32)
    Wk = np.asarray(inputs["Wk"], np.float32)
    bk = np.asarray(inputs["bk"], np.float32)
    Wv = np.asarray(inputs["Wv"], np.float32)
    bv = np.asarray(inputs["bv"], np.float32)
    Wo = np.asarray(inputs["Wo"], np.float32)
    bo = np.asarray(inputs["bo"], np.float32)
    g1 = np.asarray(inputs["g1"], np.float32)
    b1 = np.asarray(inputs["b1"], np.float32)
    g2 = np.asarray(inputs["g2"], np.float32)
    b2 = np.asarray(inputs["b2"], np.float32)
    Wc1 = np.asarray(inputs["Wc1"], np.float32)
    bc1 = np.asarray(inputs["bc1"], np.float32)
    Wc2 = np.asarray(inputs["Wc2"], np.float32)
    bc2 = np.asarray(inputs["bc2"], np.float32)

    xT = np.ascontiguousarray(x.transpose(0, 2, 1).reshape(B, NDC, 128, S)).astype(bf16)
    xp = np.ascontiguousarray((x + bo[None, None, :]).reshape(B, NSC, 128, D))

    wqk = np.zeros((2, 4, 128, 512), np.float32)  # cast to bf16 below
    for proj, W in ((0, Wq), (1, Wk)):
        for pair in range(4):
            blk = np.concatenate([W[2 * pair], W[2 * pair + 1]], axis=1)
            wqk[proj, pair] = blk.reshape(NDC, 128, 128).transpose(1, 0, 2) \
                                 .reshape(128, 512)
    bqk = np.zeros((128, 8), np.float32)
    for proj, b in ((0, bq), (1, bk)):
        for pair in range(4):
            bqk[:, proj * 4 + pair] = np.concatenate(
                [b[2 * pair], b[2 * pair + 1]])

    wv = np.zeros((NDC, 128, 520), np.float32)
    bvrow = np.zeros((128, 520), np.float32)
    for h in range(H):
        wv[:, :, h * 65:h * 65 + 64] = Wv[h].reshape(NDC, 128, 64)
        bvrow[0, h * 65:h * 65 + 64] = bv[h]
        bvrow[0, h * 65 + 64] = 1.0

    wo = np.ascontiguousarray(Wo.reshape(4, 128, 512))

    w1 = np.ascontiguousarray(
        Wc1.reshape(NCD, 128, NDC, 128, KS).transpose(0, 2, 3, 4, 1)
           .reshape(NCD, NDC, 128, KS * 128)).astype(bf16)
    # conv2 weights: fp8 pairs (chunks 0..11, x128) + bf16 (12..15, x128)
    w2full = Wc2.reshape(D, NCD, 128, KS).transpose(1, 2, 3, 0) \
                .reshape(NCD, 128, KS * 512) * S2
    w2f8 = np.ascontiguousarray(
        w2full[:2 * NF8P].reshape(NF8P, 2, 128, KS * 512)
        .transpose(0, 2, 1, 3)).astype(fp8)
    bc1s = np.ascontiguousarray(bc1.reshape(NCD, 128).T)

    # conv2 bias, prescaled by S2, + compensation for the -0.5 c1 shift on
    # the fp8 channels (0.5 * sum of their W2 taps)
    comp = 0.5 * Wc2[:, :2 * NF8P * 128, :].sum(axis=(1, 2))
    bc2s = S2 * (bc2 + comp)

    gb = np.stack([np.tile(v[None, :], (128, 1))
                   for v in (g1, b1, g2, b2, bc2s)]).astype(np.float32)
    cones = np.ones((128, 128), np.float32)
    czero = np.zeros((128, 8), bf16)
    cmh = np.full((128, 2, 8), -0.5, fp8)

    wqk = wqk.astype(bf16)
    wv = wv.astype(bf16)
    shared = dict(wqk=wqk, bqk=bqk, wv=wv, bvrow=bvrow, wo=wo,
                  w1=w1, w2f8=w2f8, bc1s=bc1s, gb=gb,
                  cones=cones, czero=czero, cmh=cmh)
    if NBF > 0:
        shared["w2bf"] = np.ascontiguousarray(w2full[2 * NF8P:]).astype(bf16)
    in_maps = []
    for c in range(NCORES):
        m = dict(shared)
        m["xT"] = np.ascontiguousarray(xT[c * NIT:(c + 1) * NIT])
        m["xp"] = np.ascontiguousarray(xp[c * NIT:(c + 1) * NIT])
        in_maps.append(m)
    return in_maps


def run(inputs, trace=False, **trace_kwargs):
    nc = _build()
    from concourse.bass_utils import run_bass_kernel_spmd
    in_maps = _prep_host(inputs)
    res = run_bass_kernel_spmd(nc, in_maps, core_ids=list(range(NCORES)),
                               trace=trace, **trace_kwargs)
    y = np.concatenate([res.results[c]["y"].reshape(NIT, S, D)
                        for c in range(NCORES)], axis=0)
    return y, res


def kernel(**inputs):
    y, _ = run(inputs, trace=False)
    return y
